# revision 44
# baseline (speedup 1.0000x reference)
"""Causal self-attention with rotary embeddings on 8 Trainium2 NeuronCores.

Tensor-parallel over heads: 16 heads / 8 cores = 2 heads per core.
Each core computes qkv for its 2 heads, rotary, causal attention, and a
partial output projection (its 128 rows of w_proj); the host sums the 8
partial outputs.

Device-side structure (per core, heads A/B local):
  - Everything "transposed": Q^T/K^T stored [d(128=A:0-63,B:64-127), t].
  - Work is emitted as a pipeline of (batch, q-chunk) units:
      scores+exp(unit) | qkv+rotary(next chunk) | PV | normalize | proj
    so TensorE always has dense matmul work while ScalarE exponentiates
    and the projection + y DMA spread across the whole kernel.
  - Scores S^T = K_blk @ Q^T -> [k(128), q], computed for both heads as
    two row-tiled matmuls (head A rows 0-63, head B rows 64-127) that
    run concurrently in the PE array (K=64 each).
  - exp per k-block over both heads in one ACTIVATE on [128, 2, 512-off]
    (off = causal column offset); diagonal 128-col block masked via one
    DVE multiply; PV matmuls use partial-N accumulation so fully-masked
    columns are never touched (no padding memsets).
  - Softmax denominator via a ones-augmented V column (extra lhsT column
    produces the k-sum row). No max-subtraction (scores are O(6)).
  - Rotary applied in the transposed layout via a pair-swap permutation
    matmul: rot(q) = cos_exp * q + sin_sgn * (Pswap @ q).
  - V transposed to t-major [k, d] tiles with the PE transpose path.

All matmul inputs fp16 (1 cyc/row on PE); accumulation fp32 in PSUM.
"""

import numpy as np

B, T, C, H = 2, 2048, 1024, 16
HD = C // H            # 64
N_CORES = 8
HPC = H // N_CORES     # 2 heads per core
BT = B * T             # 4096
TC = 512               # t-chunk (and q-chunk) size
NC_ = T // TC          # 4 chunks per batch
KB = 128               # k-block size
NKB = T // KB          # 16 k-blocks per batch
CCH = C // 128         # 8 contraction chunks

_CACHE = {}


def _build_bass():
    import concourse.bacc as bacc
    import concourse.mybir as mybir
    import concourse.tile as tile
    from concourse.masks import make_identity, make_upper_triangular

    f16 = mybir.dt.float16
    f32 = mybir.dt.float32

    nc = bacc.Bacc(num_swdge_queues=4)

    # host-prepacked layouts: per-partition-contiguous so DMA bursts are
    # 8KB (x chunks) / 6KB (wqkv) instead of scattered sub-KB rows
    xT = nc.dram_tensor("xT", [128, NC_ * B, CCH, TC], f16,
                        kind="ExternalInput")
    wqkv = nc.dram_tensor("wqkv", [128, CCH, 3 * HPC * HD], f16,
                          kind="ExternalInput")
    wp = nc.dram_tensor("wp", [HPC * HD, C], f16, kind="ExternalInput")
    cos_e = nc.dram_tensor("cos_e", [128, T], f16, kind="ExternalInput")
    sin_e = nc.dram_tensor("sin_e", [128, T], f16, kind="ExternalInput")
    pswap = nc.dram_tensor("pswap", [128, 128], f16, kind="ExternalInput")
    y = nc.dram_tensor("y", [BT, C], f16, kind="ExternalOutput")

    with tile.TileContext(nc) as tc:
        with (
            tc.tile_pool(name="const", bufs=1) as const,
            tc.tile_pool(name="persist", bufs=1) as persist,
            tc.tile_pool(name="xp", bufs=3) as xp,
            tc.tile_pool(name="rot", bufs=3) as rotp,
            tc.tile_pool(name="ptp", bufs=24) as ptp,
            tc.tile_pool(name="np_", bufs=3) as normp,
            tc.tile_pool(name="yp", bufs=2) as yp,
            tc.tile_pool(name="work", bufs=2, space="PSUM") as work,
            tc.tile_pool(name="acc2", bufs=2, space="PSUM") as acc2,
            tc.tile_pool(name="stp", bufs=2, space="PSUM") as stp,
        ):
            # ---- constants ----
            # one dma_start lands on ~one HW queue (~25 GB/s), so split
            # the startup-critical inputs into pieces across both HWDGE
            # queues (scalar: wqkv, sync: x chunk 0 in ph1) with the
            # first-needed cc slices in front.
            wqkv_sb = const.tile([128, CCH, 384], f16)
            # Q columns first: the first matmul group reads only [:, :, 0:128]
            nc.scalar.dma_start(out=wqkv_sb[:, :, 0:128],
                                in_=wqkv[:, :, 0:128])
            nc.scalar.dma_start(out=wqkv_sb[:, :, 128:384],
                                in_=wqkv[:, :, 128:384])
            cos_sb = const.tile([128, T], f16)
            sin_sb = const.tile([128, T], f16)
            pswap_sb = const.tile([128, 128], f16)
            wp_sb = const.tile([128, C], f16)
            nc.scalar.dma_start(out=cos_sb, in_=cos_e[:, :])
            nc.scalar.dma_start(out=sin_sb, in_=sin_e[:, :])
            nc.scalar.dma_start(out=pswap_sb, in_=pswap[:, :])
            nc.scalar.dma_start(out=wp_sb, in_=wp[:, :])
            ident = const.tile([128, 128], f16)
            make_identity(nc, ident)
            # mask[k, q] = 1 where q >= k (keep), 0 where q < k
            mask_ut = const.tile([128, 128], f16)
            make_upper_triangular(nc, mask_ut, val=1.0, diag=True)
            ones1 = const.tile([1, 128], f32)
            nc.gpsimd.memset(ones1, 1.0)

            # PE warmup: dependency-free matmuls on the on-chip identity
            # keep the PE HAM activity window busy while the first input
            # DMAs land, so real matmuls start at 2.4 GHz.
            warm_ps = work.tile([128, 128], f32, tag="work", name="warm")
            for _ in range(80):
                nc.tensor.matmul(warm_ps, ident, ident,
                                 start=True, stop=True)

            # ---- persistent tensors ----
            QrotT = persist.tile([128, B, T], f16)
            KrotT = persist.tile([128, B, T], f16)
            # V in t-major, per (batch, k-block): [V_A(64) | ones | V_B(64) | ones]
            Vaug = persist.tile([128, B, NKB, 130], f16)
            Yn = persist.tile([128, B, T], f16)
            ones_cols = Vaug.rearrange(
                "p b J (h x) -> p b J h x", x=65)[:, :, :, :, 64]
            nc.gpsimd.memset(ones_cols, 1.0)

            # ============ phase-1 chunk: qkv + rotary + V transpose ========
            def ph1_pieces(b, i):
                """Return emission closures for t-chunk i of batch b."""
                ci = b * NC_ + i
                cs = slice(i * TC, (i + 1) * TC)
                state = {}

                def dma_x():
                    x_sb = xp.tile([128, CCH, TC], f16, tag="x")
                    state["x"] = x_sb
                    if ci == 0:
                        nc.sync.dma_start(out=x_sb[:, 0:4, :],
                                          in_=xT[:, 0, 0:4, :])
                        nc.gpsimd.dma_start(out=x_sb[:, 4:8, :],
                                            in_=xT[:, 0, 4:8, :])
                    elif ci % 2 == 0:
                        nc.gpsimd.dma_start(out=x_sb, in_=xT[:, ci, :, :])
                    else:
                        nc.sync.dma_start(out=x_sb, in_=xT[:, ci, :, :])

                def qk_group(g):
                    x_sb = state["x"]
                    dst = QrotT if g == 0 else KrotT
                    acc = work.tile([128, TC], f32, tag="work", name="acc")
                    for cc in range(CCH):
                        nc.tensor.matmul(
                            acc, wqkv_sb[:, cc, g * 128:(g + 1) * 128],
                            x_sb[:, cc, :],
                            start=(cc == 0), stop=(cc == CCH - 1))
                    graw = rotp.tile([128, TC], f16, tag="graw")
                    nc.vector.tensor_copy(graw, acc)
                    swp = work.tile([128, TC], f32, tag="work", name="swp")
                    nc.tensor.matmul(swp, pswap_sb, graw,
                                     start=True, stop=True)
                    t1 = rotp.tile([128, TC], f16, tag="t1")
                    nc.vector.tensor_mul(t1, graw, cos_sb[:, cs])
                    t2 = rotp.tile([128, TC], f16, tag="t2")
                    nc.vector.tensor_mul(t2, swp, sin_sb[:, cs])
                    nc.vector.tensor_add(dst[:, b, cs], t1, t2)

                def v_group():
                    x_sb = state["x"]
                    acc = work.tile([128, TC], f32, tag="work", name="vacc")
                    for cc in range(CCH):
                        nc.tensor.matmul(
                            acc, wqkv_sb[:, cc, 256:384], x_sb[:, cc, :],
                            start=(cc == 0), stop=(cc == CCH - 1))
                    vtmp = rotp.tile([128, TC], f16, tag="vtmp")
                    nc.vector.tensor_copy(vtmp, acc)
                    state["vtmp"] = vtmp

                def v_trans():
                    vtmp = state["vtmp"]
                    for q in range(TC // 128):
                        J = i * (TC // 128) + q
                        vt = work.tile([128, 128], f16, tag="work", name="vt")
                        nc.tensor.transpose(
                            vt, vtmp[:, q * 128:(q + 1) * 128], ident)
                        vdst = Vaug[:, b, J, :].rearrange(
                            "p (h x) -> p h x", x=65)[:, :, 0:64]
                        vsrc = vt.rearrange("p (h x) -> p h x", h=2)
                        nc.vector.tensor_copy(vdst, vsrc)

                return [dma_x, lambda: qk_group(0), lambda: qk_group(1),
                        v_group, v_trans]

            # ============ phase-2 unit: attention for (b, q-chunk c) =======
            def emit_unit(b, c, next_pieces):
                jmax = 4 * c + 3
                qs0 = c * TC          # chunk start within batch
                pts = []
                pieces_done = 0
                ypss = [acc2.tile([128, TC], f32, tag="acc2", name="yps")
                        for _ in range(2)]

                def pv(j):
                    pt, off = pts[j]
                    for h in range(2):
                        nc.tensor.matmul(
                            ypss[h][0:65, off:TC],
                            Vaug[:, b, j, h * 65:(h + 1) * 65],
                            pt[:, h, off:TC],
                            start=(j == 0), stop=(j == jmax))

                for j in range(jmax + 1):
                    off = max(0, j * KB - qs0)  # valid col offset in chunk
                    st = stp.tile([128, 2, TC], f32, tag="st", name="st")
                    for h in range(2):
                        hs = slice(h * 64, (h + 1) * 64)
                        nc.tensor.matmul(
                            st[:, h, off:TC],
                            KrotT[hs, b, j * KB:(j + 1) * KB],
                            QrotT[hs, b, qs0 + off:qs0 + TC],
                            start=True, stop=True)
                    pt = ptp.tile([128, 2, TC], f16, tag="pt", name="pt")
                    nc.scalar.activation(
                        pt[:, :, off:TC], st[:, :, off:TC],
                        mybir.ActivationFunctionType.Exp)
                    if j >= 4 * c:  # diagonal band: triangular mask
                        for h in range(2):
                            nc.vector.tensor_mul(
                                pt[:, h, off:off + 128],
                                pt[:, h, off:off + 128], mask_ut)
                    pts.append((pt, off))
                    # PV trails scores by 2 so exp/mask have drained
                    if j >= 2:
                        pv(j - 2)
                    # interleave next chunk's qkv work into the PE stream
                    want = (len(next_pieces) * (j + 1)) // (jmax + 1)
                    while pieces_done < want:
                        next_pieces[pieces_done]()
                        pieces_done += 1
                while pieces_done < len(next_pieces):
                    next_pieces[pieces_done]()
                    pieces_done += 1
                pv(jmax - 1)
                pv(jmax)
                if (b, c) == (1, 3):
                    # tail filler: keep the PE HAM-warm through the last
                    # unit's normalize chain so the projection runs at
                    # full clock
                    tail_ps = work.tile([128, 128], f32, tag="work",
                                        name="tail_ps")
                    for _ in range(30):
                        nc.tensor.matmul(tail_ps, ident, ident,
                                         start=True, stop=True)

                # ---- normalize: rows 0-63 divided by the ones-row (64) ----
                # custom-DVE reciprocal misreads PSUM/cross-partition inputs,
                # so stage both heads' denominators into SBUF partition 0.
                dsb = normp.tile([1, 2, TC], f32, tag="dsb")
                for h in range(2):
                    nc.vector.tensor_copy(dsb[0:1, h, :], ypss[h][64:65, :])
                rec = normp.tile([1, 2, TC], f32, tag="rec")
                nc.vector.reciprocal_approx_fast(
                    out=rec.rearrange("p a x -> p (a x)"),
                    in_=dsb.rearrange("p a x -> p (a x)"))
                cslice = slice(qs0, qs0 + TC)
                for h in range(2):
                    bc = normp.tile([64, TC], f32, tag="bc", name="bc")
                    nc.gpsimd.partition_broadcast(bc, rec[0:1, h, :])
                    if h == 0:
                        nc.vector.tensor_tensor(
                            out=Yn[0:64, b, cslice],
                            in0=ypss[h][0:64, :], in1=bc,
                            op=mybir.AluOpType.mult)
                    else:
                        ytmp = normp.tile([64, TC], f16, tag="ytmp")
                        nc.vector.tensor_tensor(
                            out=ytmp, in0=ypss[h][0:64, :], in1=bc,
                            op=mybir.AluOpType.mult)
                        # cross-partition move 0-63 -> 64-127 via DMA
                        nc.scalar.dma_start(
                            out=Yn[64:128, b, cslice], in_=ytmp)

                # ---- projection for this q-chunk ----
                # y writes ride the gpsimd SWDGE queue (one batched DMA
                # per unit) so they don't contend with the x input stream
                # on the sync HWDGE queue; the last unit drains per-tt
                # across all three queues instead.
                last = (b, c) == (1, 3)
                r0 = b * T + qs0
                yout = yp.tile([128, 4, C], f16, tag="yo", name="yout")
                for tt in range(4):
                    for half in range(2):
                        pout = acc2.tile([128, 512], f32, tag="acc2",
                                         name="pout")
                        nc.tensor.matmul(
                            pout,
                            Yn[:, b, qs0 + tt * 128:qs0 + (tt + 1) * 128],
                            wp_sb[:, half * 512:(half + 1) * 512],
                            start=True, stop=True)
                        dstap = yout[:, tt, half * 512:(half + 1) * 512]
                        if (tt * 2 + half) % 4 == 3:
                            nc.scalar.copy(dstap, pout)
                        else:
                            nc.vector.tensor_copy(dstap, pout)
                    if last:
                        eng = (nc.gpsimd, nc.sync, nc.scalar, nc.sync)[tt]
                        eng.dma_start(
                            out=y[r0 + tt * 128:r0 + (tt + 1) * 128, :],
                            in_=yout[:, tt, :])
                if not last:
                    yeng = nc.gpsimd if (b * NC_ + c) % 2 == 0 else nc.scalar
                    yeng.dma_start(
                        out=y[r0:r0 + TC, :].rearrange(
                            "(tt p) c -> p tt c", p=128),
                        in_=yout)

            # ================= emission: pipelined units ===================
            units = [(b, c) for b in range(B) for c in range(NC_)]
            feeds = {u: [units[u + 1]] for u in range(len(units) - 1)}
            for fn in ph1_pieces(0, 0):
                fn()
            for u, (b, c) in enumerate(units):
                nxt = []
                for fb, fc in feeds.get(u, []):
                    nxt.extend(ph1_pieces(fb, fc))
                emit_unit(b, c, nxt)

    nc.finalize()
    return nc


def _host_prep(x, cos, sin, w_attn, b_attn, w_proj):
    """Shared + per-core input arrays (all fp16 except noted)."""
    x2 = np.asarray(x, dtype=np.float32).reshape(BT, C)
    xT16 = np.ascontiguousarray(x2.T).astype(np.float16)
    # repack [C, BT] -> [p, chunk, cc, t] so each DMA chunk reads one
    # contiguous 8KB run per partition
    xprep = np.ascontiguousarray(
        xT16.reshape(CCH, 128, B * NC_, TC).transpose(1, 2, 0, 3))

    cos = np.asarray(cos, dtype=np.float32)
    sin = np.asarray(sin, dtype=np.float32)
    d = np.arange(128) % 64
    freq_i = d // 2
    sign = np.where(d % 2 == 0, -1.0, 1.0).astype(np.float32)
    cos_exp = cos[:, freq_i].T.astype(np.float16)               # [128, T]
    sin_exp = (sign[:, None] * sin[:, freq_i].T).astype(np.float16)

    pswap = np.zeros((128, 128), dtype=np.float16)
    idx = np.arange(128)
    pswap[idx ^ 1, idx] = 1.0

    w_attn = np.asarray(w_attn, dtype=np.float32)
    w_proj = np.asarray(w_proj, dtype=np.float32)
    scale = 1.0 / np.sqrt(HD)

    per_core = []
    for m in range(N_CORES):
        cols = []
        for g in range(3):          # q, k, v blocks of w_attn
            for hh in range(HPC):
                hglob = m * HPC + hh
                blk = w_attn[:, g * C + hglob * HD:(g * C + (hglob + 1) * HD)]
                if g == 0:
                    blk = blk * scale
                cols.append(blk)
        w_stack = np.concatenate(cols, axis=1).astype(np.float16)
        # repack [C, 384] -> [p, cc, j] (contiguous per partition)
        w_stack = np.ascontiguousarray(
            w_stack.reshape(CCH, 128, 384).transpose(1, 0, 2))
        wp_m = w_proj[m * HPC * HD:(m + 1) * HPC * HD, :].astype(np.float16)
        per_core.append((w_stack, wp_m))
    return xprep, cos_exp, sin_exp, pswap, per_core


def kernel(x, cos, sin, w_attn, b_attn, w_proj, b_proj):
    from concourse.bass_utils import run_bass_kernel_spmd

    b_attn = np.asarray(b_attn, dtype=np.float32)
    assert not np.any(b_attn), "nonzero b_attn not supported by this kernel"

    xT16, cos_exp, sin_exp, pswap, per_core = _host_prep(
        x, cos, sin, w_attn, b_attn, w_proj)

    if "nc" not in _CACHE:
        _CACHE["nc"] = _build_bass()
    nc = _CACHE["nc"]

    in_maps = []
    for m in range(N_CORES):
        w_stack, wp_m = per_core[m]
        in_maps.append({
            "xT": xT16, "wqkv": w_stack, "wp": wp_m,
            "cos_e": cos_exp, "sin_e": sin_exp, "pswap": pswap,
        })

    res = run_bass_kernel_spmd(nc, in_maps, core_ids=list(range(N_CORES)))
    _CACHE["last_result"] = res

    y = np.zeros((BT, C), dtype=np.float64)
    for m in range(N_CORES):
        y += res.results[m]["y"].astype(np.float64)
    y = y + np.asarray(b_proj, dtype=np.float64)[None, :]
    return y.reshape(B, T, C).astype(np.float32)


# revision 46
# speedup vs baseline: 1.0365x; 1.0365x over previous
"""Causal self-attention with rotary embeddings on 8 Trainium2 NeuronCores.

Tensor-parallel over heads: 16 heads / 8 cores = 2 heads per core.
Each core computes qkv for its 2 heads, rotary, causal attention, and a
partial output projection (its 128 rows of w_proj); the host sums the 8
partial outputs.

Device-side structure (per core, heads A/B local):
  - Everything "transposed": Q^T/K^T stored [d(128=A:0-63,B:64-127), t].
  - Work is emitted as a pipeline of (batch, q-chunk) units:
      scores+exp(unit) | qkv+rotary(next chunk) | PV | normalize | proj
    so TensorE always has dense matmul work while ScalarE exponentiates
    and the projection + y DMA spread across the whole kernel.
  - Scores S^T = K_blk @ Q^T -> [k(128), q], computed for both heads as
    two row-tiled matmuls (head A rows 0-63, head B rows 64-127) that
    run concurrently in the PE array (K=64 each).
  - exp per k-block over both heads in one ACTIVATE on [128, 2, 512-off]
    (off = causal column offset); diagonal 128-col block masked via one
    DVE multiply; PV matmuls use partial-N accumulation so fully-masked
    columns are never touched (no padding memsets).
  - Softmax denominator via a ones-augmented V column (extra lhsT column
    produces the k-sum row). No max-subtraction (scores are O(6)).
  - Rotary applied in the transposed layout via a pair-swap permutation
    matmul: rot(q) = cos_exp * q + sin_sgn * (Pswap @ q).
  - V transposed to t-major [k, d] tiles with the PE transpose path.

All matmul inputs fp16 (1 cyc/row on PE); accumulation fp32 in PSUM.
"""

import numpy as np

B, T, C, H = 2, 2048, 1024, 16
HD = C // H            # 64
N_CORES = 8
HPC = H // N_CORES     # 2 heads per core
BT = B * T             # 4096
TC = 512               # t-chunk (and q-chunk) size
NC_ = T // TC          # 4 chunks per batch
KB = 128               # k-block size
NKB = T // KB          # 16 k-blocks per batch
CCH = C // 128         # 8 contraction chunks

_CACHE = {}


def _build_bass():
    import concourse.bacc as bacc
    import concourse.mybir as mybir
    import concourse.tile as tile
    from concourse.masks import make_identity, make_upper_triangular

    f16 = mybir.dt.float16
    f32 = mybir.dt.float32

    nc = bacc.Bacc(num_swdge_queues=4)

    # host-prepacked layouts: per-partition-contiguous so DMA bursts are
    # 8KB (x chunks) / 6KB (wqkv) instead of scattered sub-KB rows
    xT = nc.dram_tensor("xT", [128, NC_ * B, CCH, TC], f16,
                        kind="ExternalInput")
    wqkv = nc.dram_tensor("wqkv", [128, CCH, 3 * HPC * HD], f16,
                          kind="ExternalInput")
    wp = nc.dram_tensor("wp", [HPC * HD, C], f16, kind="ExternalInput")
    cos_e = nc.dram_tensor("cos_e", [128, T], f16, kind="ExternalInput")
    sin_e = nc.dram_tensor("sin_e", [128, T], f16, kind="ExternalInput")
    pswap = nc.dram_tensor("pswap", [128, 128], f16, kind="ExternalInput")
    y = nc.dram_tensor("y", [BT, C], f16, kind="ExternalOutput")

    with tile.TileContext(nc) as tc:
        with (
            tc.tile_pool(name="const", bufs=1) as const,
            tc.tile_pool(name="persist", bufs=1) as persist,
            tc.tile_pool(name="xp", bufs=3) as xp,
            tc.tile_pool(name="rot", bufs=3) as rotp,
            tc.tile_pool(name="ptp", bufs=24) as ptp,
            tc.tile_pool(name="np_", bufs=3) as normp,
            tc.tile_pool(name="yp", bufs=2) as yp,
            tc.tile_pool(name="work", bufs=2, space="PSUM") as work,
            tc.tile_pool(name="acc2", bufs=2, space="PSUM") as acc2,
            tc.tile_pool(name="stp", bufs=2, space="PSUM") as stp,
        ):
            # ---- constants ----
            # one dma_start lands on ~one HW queue (~25 GB/s), so split
            # the startup-critical inputs into pieces across both HWDGE
            # queues (scalar: wqkv, sync: x chunk 0 in ph1) with the
            # first-needed cc slices in front.
            wqkv_sb = const.tile([128, CCH, 384], f16)
            # Q columns first: the first matmul group reads only [:, :, 0:128]
            nc.scalar.dma_start(out=wqkv_sb[:, :, 0:128],
                                in_=wqkv[:, :, 0:128])
            nc.scalar.dma_start(out=wqkv_sb[:, :, 128:384],
                                in_=wqkv[:, :, 128:384])
            cos_sb = const.tile([128, T], f16)
            sin_sb = const.tile([128, T], f16)
            pswap_sb = const.tile([128, 128], f16)
            wp_sb = const.tile([128, C], f16)
            nc.scalar.dma_start(out=cos_sb, in_=cos_e[:, :])
            nc.scalar.dma_start(out=sin_sb, in_=sin_e[:, :])
            nc.scalar.dma_start(out=pswap_sb, in_=pswap[:, :])
            nc.scalar.dma_start(out=wp_sb, in_=wp[:, :])
            ident = const.tile([128, 128], f16)
            make_identity(nc, ident)
            # mask[k, q] = 1 where q >= k (keep), 0 where q < k
            mask_ut = const.tile([128, 128], f16)
            make_upper_triangular(nc, mask_ut, val=1.0, diag=True)
            ones1 = const.tile([1, 128], f32)
            nc.gpsimd.memset(ones1, 1.0)

            # PE warmup: dependency-free matmuls on the on-chip identity
            # keep the PE HAM activity window busy while the first input
            # DMAs land, so real matmuls start at 2.4 GHz.
            warm_ps = work.tile([128, 128], f32, tag="work", name="warm")
            for _ in range(40):
                nc.tensor.matmul(warm_ps, ident, ident,
                                 start=True, stop=True)

            # ---- persistent tensors ----
            QrotT = persist.tile([128, B, T], f16)
            KrotT = persist.tile([128, B, T], f16)
            # V in t-major, per (batch, k-block): [V_A(64) | ones | V_B(64) | ones]
            Vaug = persist.tile([128, B, NKB, 130], f16)
            Yn = persist.tile([128, B, T], f16)
            ones_cols = Vaug.rearrange(
                "p b J (h x) -> p b J h x", x=65)[:, :, :, :, 64]
            nc.gpsimd.memset(ones_cols, 1.0)

            # ============ phase-1 chunk: qkv + rotary + V transpose ========
            def ph1_pieces(b, i):
                """Return emission closures for t-chunk i of batch b."""
                ci = b * NC_ + i
                cs = slice(i * TC, (i + 1) * TC)
                state = {}

                def dma_x():
                    x_sb = xp.tile([128, CCH, TC], f16, tag="x")
                    state["x"] = x_sb
                    if ci == 0:
                        nc.sync.dma_start(out=x_sb[:, 0:4, :],
                                          in_=xT[:, 0, 0:4, :])
                        nc.gpsimd.dma_start(out=x_sb[:, 4:8, :],
                                            in_=xT[:, 0, 4:8, :])
                    else:
                        nc.sync.dma_start(out=x_sb, in_=xT[:, ci, :, :])

                def qk_group(g):
                    x_sb = state["x"]
                    dst = QrotT if g == 0 else KrotT
                    acc = work.tile([128, TC], f32, tag="work", name="acc")
                    for cc in range(CCH):
                        nc.tensor.matmul(
                            acc, wqkv_sb[:, cc, g * 128:(g + 1) * 128],
                            x_sb[:, cc, :],
                            start=(cc == 0), stop=(cc == CCH - 1))
                    graw = rotp.tile([128, TC], f16, tag="graw")
                    nc.vector.tensor_copy(graw, acc)
                    swp = work.tile([128, TC], f32, tag="work", name="swp")
                    nc.tensor.matmul(swp, pswap_sb, graw,
                                     start=True, stop=True)
                    t1 = rotp.tile([128, TC], f16, tag="t1")
                    nc.vector.tensor_mul(t1, graw, cos_sb[:, cs])
                    t2 = rotp.tile([128, TC], f16, tag="t2")
                    nc.vector.tensor_mul(t2, swp, sin_sb[:, cs])
                    nc.vector.tensor_add(dst[:, b, cs], t1, t2)

                def v_group():
                    x_sb = state["x"]
                    acc = work.tile([128, TC], f32, tag="work", name="vacc")
                    for cc in range(CCH):
                        nc.tensor.matmul(
                            acc, wqkv_sb[:, cc, 256:384], x_sb[:, cc, :],
                            start=(cc == 0), stop=(cc == CCH - 1))
                    vtmp = rotp.tile([128, TC], f16, tag="vtmp")
                    nc.vector.tensor_copy(vtmp, acc)
                    state["vtmp"] = vtmp

                def v_trans():
                    vtmp = state["vtmp"]
                    for q in range(TC // 128):
                        J = i * (TC // 128) + q
                        vt = work.tile([128, 128], f16, tag="work", name="vt")
                        nc.tensor.transpose(
                            vt, vtmp[:, q * 128:(q + 1) * 128], ident)
                        vdst = Vaug[:, b, J, :].rearrange(
                            "p (h x) -> p h x", x=65)[:, :, 0:64]
                        vsrc = vt.rearrange("p (h x) -> p h x", h=2)
                        nc.vector.tensor_copy(vdst, vsrc)

                return [dma_x, lambda: qk_group(0), lambda: qk_group(1),
                        v_group, v_trans]

            # ============ phase-2 unit: attention for (b, q-chunk c) =======
            def emit_unit(b, c, next_pieces):
                jmax = 4 * c + 3
                qs0 = c * TC          # chunk start within batch
                pts = []
                pieces_done = 0
                ypss = [acc2.tile([128, TC], f32, tag="acc2", name="yps")
                        for _ in range(2)]

                def pv(j):
                    pt, off = pts[j]
                    for h in range(2):
                        nc.tensor.matmul(
                            ypss[h][0:65, off:TC],
                            Vaug[:, b, j, h * 65:(h + 1) * 65],
                            pt[:, h, off:TC],
                            start=(j == 0), stop=(j == jmax))

                for j in range(jmax + 1):
                    off = max(0, j * KB - qs0)  # valid col offset in chunk
                    st = stp.tile([128, 2, TC], f32, tag="st", name="st")
                    for h in range(2):
                        hs = slice(h * 64, (h + 1) * 64)
                        nc.tensor.matmul(
                            st[:, h, off:TC],
                            KrotT[hs, b, j * KB:(j + 1) * KB],
                            QrotT[hs, b, qs0 + off:qs0 + TC],
                            start=True, stop=True)
                    pt = ptp.tile([128, 2, TC], f16, tag="pt", name="pt")
                    nc.scalar.activation(
                        pt[:, :, off:TC], st[:, :, off:TC],
                        mybir.ActivationFunctionType.Exp)
                    if j >= 4 * c:  # diagonal band: triangular mask
                        for h in range(2):
                            nc.vector.tensor_mul(
                                pt[:, h, off:off + 128],
                                pt[:, h, off:off + 128], mask_ut)
                    pts.append((pt, off))
                    # PV trails scores by 2 so exp/mask have drained
                    if j >= 2:
                        pv(j - 2)
                    # interleave next chunk's qkv work into the PE stream
                    want = (len(next_pieces) * (j + 1)) // (jmax + 1)
                    while pieces_done < want:
                        next_pieces[pieces_done]()
                        pieces_done += 1
                while pieces_done < len(next_pieces):
                    next_pieces[pieces_done]()
                    pieces_done += 1
                pv(jmax - 1)
                pv(jmax)
                if (b, c) == (1, 3):
                    # tail filler: keep the PE HAM-warm through the last
                    # unit's normalize chain so the projection runs at
                    # full clock
                    tail_ps = work.tile([128, 128], f32, tag="work",
                                        name="tail_ps")
                    for _ in range(30):
                        nc.tensor.matmul(tail_ps, ident, ident,
                                         start=True, stop=True)

                # ---- normalize: rows 0-63 divided by the ones-row (64) ----
                # custom-DVE reciprocal misreads PSUM/cross-partition inputs,
                # so stage both heads' denominators into SBUF partition 0.
                dsb = normp.tile([1, 2, TC], f32, tag="dsb")
                for h in range(2):
                    nc.vector.tensor_copy(dsb[0:1, h, :], ypss[h][64:65, :])
                rec = normp.tile([1, 2, TC], f32, tag="rec")
                nc.vector.reciprocal_approx_fast(
                    out=rec.rearrange("p a x -> p (a x)"),
                    in_=dsb.rearrange("p a x -> p (a x)"))
                cslice = slice(qs0, qs0 + TC)
                for h in range(2):
                    bc = normp.tile([64, TC], f32, tag="bc", name="bc")
                    nc.gpsimd.partition_broadcast(bc, rec[0:1, h, :])
                    if h == 0:
                        nc.vector.tensor_tensor(
                            out=Yn[0:64, b, cslice],
                            in0=ypss[h][0:64, :], in1=bc,
                            op=mybir.AluOpType.mult)
                    else:
                        ytmp = normp.tile([64, TC], f16, tag="ytmp")
                        nc.vector.tensor_tensor(
                            out=ytmp, in0=ypss[h][0:64, :], in1=bc,
                            op=mybir.AluOpType.mult)
                        # cross-partition move 0-63 -> 64-127 via DMA
                        nc.scalar.dma_start(
                            out=Yn[64:128, b, cslice], in_=ytmp)

                # ---- projection for this q-chunk ----
                # y writes ride the gpsimd SWDGE queue (one batched DMA
                # per unit) so they don't contend with the x input stream
                # on the sync HWDGE queue; the last unit drains per-tt
                # across all three queues instead.
                last = (b, c) == (1, 3)
                r0 = b * T + qs0
                yout = yp.tile([128, 4, C], f16, tag="yo", name="yout")
                for tt in range(4):
                    for half in range(2):
                        pout = acc2.tile([128, 512], f32, tag="acc2",
                                         name="pout")
                        nc.tensor.matmul(
                            pout,
                            Yn[:, b, qs0 + tt * 128:qs0 + (tt + 1) * 128],
                            wp_sb[:, half * 512:(half + 1) * 512],
                            start=True, stop=True)
                        dstap = yout[:, tt, half * 512:(half + 1) * 512]
                        if (tt * 2 + half) % 4 == 3:
                            nc.scalar.copy(dstap, pout)
                        else:
                            nc.vector.tensor_copy(dstap, pout)
                    if last:
                        eng = (nc.gpsimd, nc.sync, nc.scalar, nc.sync)[tt]
                        eng.dma_start(
                            out=y[r0 + tt * 128:r0 + (tt + 1) * 128, :],
                            in_=yout[:, tt, :])
                if not last:
                    yeng = nc.gpsimd if (b * NC_ + c) % 2 == 0 else nc.scalar
                    yeng.dma_start(
                        out=y[r0:r0 + TC, :].rearrange(
                            "(tt p) c -> p tt c", p=128),
                        in_=yout)

            # ================= emission: pipelined units ===================
            units = [(b, c) for b in range(B) for c in range(NC_)]
            feeds = {u: [units[u + 1]] for u in range(len(units) - 1)}
            for fn in ph1_pieces(0, 0):
                fn()
            for u, (b, c) in enumerate(units):
                nxt = []
                for fb, fc in feeds.get(u, []):
                    nxt.extend(ph1_pieces(fb, fc))
                emit_unit(b, c, nxt)

    nc.finalize()
    return nc


def _host_prep(x, cos, sin, w_attn, b_attn, w_proj):
    """Shared + per-core input arrays (all fp16 except noted)."""
    x2 = np.asarray(x, dtype=np.float32).reshape(BT, C)
    xT16 = np.ascontiguousarray(x2.T).astype(np.float16)
    # repack [C, BT] -> [p, chunk, cc, t] so each DMA chunk reads one
    # contiguous 8KB run per partition
    xprep = np.ascontiguousarray(
        xT16.reshape(CCH, 128, B * NC_, TC).transpose(1, 2, 0, 3))

    cos = np.asarray(cos, dtype=np.float32)
    sin = np.asarray(sin, dtype=np.float32)
    d = np.arange(128) % 64
    freq_i = d // 2
    sign = np.where(d % 2 == 0, -1.0, 1.0).astype(np.float32)
    cos_exp = cos[:, freq_i].T.astype(np.float16)               # [128, T]
    sin_exp = (sign[:, None] * sin[:, freq_i].T).astype(np.float16)

    pswap = np.zeros((128, 128), dtype=np.float16)
    idx = np.arange(128)
    pswap[idx ^ 1, idx] = 1.0

    w_attn = np.asarray(w_attn, dtype=np.float32)
    w_proj = np.asarray(w_proj, dtype=np.float32)
    scale = 1.0 / np.sqrt(HD)

    per_core = []
    for m in range(N_CORES):
        cols = []
        for g in range(3):          # q, k, v blocks of w_attn
            for hh in range(HPC):
                hglob = m * HPC + hh
                blk = w_attn[:, g * C + hglob * HD:(g * C + (hglob + 1) * HD)]
                if g == 0:
                    blk = blk * scale
                cols.append(blk)
        w_stack = np.concatenate(cols, axis=1).astype(np.float16)
        # repack [C, 384] -> [p, cc, j] (contiguous per partition)
        w_stack = np.ascontiguousarray(
            w_stack.reshape(CCH, 128, 384).transpose(1, 0, 2))
        wp_m = w_proj[m * HPC * HD:(m + 1) * HPC * HD, :].astype(np.float16)
        per_core.append((w_stack, wp_m))
    return xprep, cos_exp, sin_exp, pswap, per_core


def kernel(x, cos, sin, w_attn, b_attn, w_proj, b_proj):
    from concourse.bass_utils import run_bass_kernel_spmd

    b_attn = np.asarray(b_attn, dtype=np.float32)
    assert not np.any(b_attn), "nonzero b_attn not supported by this kernel"

    xT16, cos_exp, sin_exp, pswap, per_core = _host_prep(
        x, cos, sin, w_attn, b_attn, w_proj)

    if "nc" not in _CACHE:
        _CACHE["nc"] = _build_bass()
    nc = _CACHE["nc"]

    in_maps = []
    for m in range(N_CORES):
        w_stack, wp_m = per_core[m]
        in_maps.append({
            "xT": xT16, "wqkv": w_stack, "wp": wp_m,
            "cos_e": cos_exp, "sin_e": sin_exp, "pswap": pswap,
        })

    res = run_bass_kernel_spmd(nc, in_maps, core_ids=list(range(N_CORES)))
    _CACHE["last_result"] = res

    y = np.zeros((BT, C), dtype=np.float64)
    for m in range(N_CORES):
        y += res.results[m]["y"].astype(np.float64)
    y = y + np.asarray(b_proj, dtype=np.float64)[None, :]
    return y.reshape(B, T, C).astype(np.float32)
